# revision 43
# baseline (speedup 1.0000x reference)
"""EnhancedAdaptiveLoRAPooling fused kernel for 8x Trainium2 NeuronCores.

Strategy v16: host-side routing + fp8 low-rank delta device kernel.

The reference output is y = x + delta(x) where delta is a rank-16 linear
map (current-task LoRA fused with the similarity-pooled LoRA).  All the
routing math (cosine/euclid sims, 4-layer MLP, top-3 + threshold,
weighted pooling, fusion weights) involves only KB-sized tensors, so it
runs on the host in f32 numpy and folds into two small matrices:
  Acomb [16, H] = [(1-fw)*S*A_cur ; fw*S*pooled_a]   (fp8, x32 scale)
  Bcomb [H, 16] = [B_cur | pooled_b]                 (bf16, scaled so
                                                      PSUM == int8 grid)
The device does only the O(B*S*H) work per 512-token tile:
  in:  xP fp8 tile-major [tile, p, c, t] (3 KiB descriptors)
  v   = Acomb^T x          fp8 DoubleRow matmuls (2 k-tiles each)
  d   = Bcomb^T v          bf16 matmuls (K zero-padded to 128: small-K
                           matmuls run at half the column rate)
  out: yP int8 tile-major  (PSUM f32 -> int8 copies split DVE/ACT)
The host adds y = x + S_D * delta in f32 (x stays exact; only the tiny
delta carries fp8/int8 noise; measured end-to-end rel err ~7e-4).

Schedule notes:
  - tile-major packing gives 3 KiB DMA descriptors: first tile lands
    ~5us sooner and the final store drains in ~1us.
  - x tiles alternate sync/scalar rings (two queues per DMA engine);
    stores go on gpsimd (+sync for the last tile).  SWDGE descriptor
    generation (~1us per DMA) executes ON the issuing engine, so no
    DMA is ever issued from a conversion engine mid-kernel.
  - v tiles interleave two tiles ahead of delta so the PSUM->SBUF v
    copies sit in the conversion-engine queues before PE needs them.
  - persistent v buffers rows 32-127 zeroed once: the delta contraction
    is K=128 (full column rate) while A is only padded to M=32 (LD time).
  - PE warmup burst ramps the clock while x0 is in flight.
"""

import numpy as np

B, S, H = 8, 4096, 768
N_TASKS, R = 16, 8
SCALING = 2.0
TOP_K = 3
NCORES = 8
TPC = (B * S) // NCORES          # tokens per core = 4096
TTK = 512                        # tokens per tile
NT = TPC // TTK                  # 8 tiles
NCH = H // 128                   # 6 hidden chunks

KA = 32.0                        # fp8 scale for Acomb
S_D = 0.5 / 127                  # int8 delta grid
_NBLOB = 48 + 384                # A2 fp8 (192B, M padded to 32) + Bcomb bf16

_PROGRAM = None


def _build_program():
    from contextlib import ExitStack

    import concourse.bass as bass  # noqa: F401
    import concourse.tile as tile
    from concourse import bacc, mybir

    f32 = mybir.dt.float32
    bf16 = mybir.dt.bfloat16
    fp8 = mybir.dt.float8e4
    i8 = mybir.dt.int8
    DR = mybir.MatmulPerfMode.DoubleRow

    nc = bacc.Bacc("TRN2", target_bir_lowering=False, debug=False)

    # tile-major x: row n*128+p holds tile n, partition p, cols (c, t)
    xP = nc.dram_tensor("xP", [NT * 128, H], f32, kind="ExternalInput").ap()
    wblob = nc.dram_tensor("wblob", [128, _NBLOB], f32,
                           kind="ExternalInput").ap()
    yP = nc.dram_tensor("yP", [NT * 128, NCH * TTK], i8,
                        kind="ExternalOutput").ap()

    xP_r = xP.rearrange("(n p) w -> p n w", p=128)
    yP_r = yP.rearrange("(n p) w -> p n w", p=128)

    with tile.TileContext(nc) as tc:
        with ExitStack() as ctx:
            const = ctx.enter_context(tc.tile_pool(name="const", bufs=1))
            wblob_sb = const.tile([128, _NBLOB], f32, name="wblob_sb")
            nc.sync.dma_start(out=wblob_sb, in_=wblob)
            # A2 [128, kp, i, 32] fp8 (DoubleRow stationary, 3 k-pairs;
            # stationary cols 16-31 zero)
            A2_sb = wblob_sb[:, 0:48].bitcast(fp8).rearrange(
                "p (k i m) -> p k i m", k=3, i=2)
            # Bc [128, c, 128] bf16; rows k<16 hold Bcomb rank k, rest zero
            Bc_sb = wblob_sb[:, 48:48 + 384].bitcast(bf16).rearrange(
                "p (c m) -> p c m", c=6)

            # x tiles all on the sync ring (3 KiB descriptors; FIFO gives
            # sequential arrival; no DMA ever issues from a conv engine)
            xp = ctx.enter_context(tc.tile_pool(name="xp", bufs=NT))
            xts = []
            for n in range(NT):
                xt = xp.tile([128, H], f32, tag="xt", name=f"xt{n}")
                nc.sync.dma_start(out=xt, in_=xP_r[:, n, :])
                xts.append(xt.bitcast(fp8).rearrange(
                    "p (c t) -> p c t", c=NCH))   # [128, 6, 512]

            vp = ctx.enter_context(tc.tile_pool(name="vp", bufs=2, space="PSUM"))
            dp = ctx.enter_context(tc.tile_pool(name="dp", bufs=6, space="PSUM"))
            yp = ctx.enter_context(tc.tile_pool(name="yp", bufs=4))

            yts = [yp.tile([128, NCH, TTK], i8, tag="yt", name=f"yt{n}")
                   for n in range(NT)]

            # persistent v buffers; rows 32-127 zeroed once (they multiply
            # zero B rows; K=128 keeps delta at full column rate)
            vpers = [const.tile([128, TTK], bf16, name=f"vbuf{j}")
                     for j in range(3)]
            for j in range(3):
                for p0 in range(32, 128, 32):
                    nc.gpsimd.memset(vpers[j][p0:p0 + 32, :], 0)

            v_sbs = {}

            def emit_vq(n):
                """v[32, TTK] = Acomb^T x for tile n + PSUM->SBUF copy."""
                xt = xts[n]
                v_ps = vp.tile([32, TTK], f32, tag="v", name="v_ps")
                v_sb = vpers[n % 3]
                v_sbs[n] = v_sb
                for q in range(2):
                    o0 = q * 256
                    for kp in range(3):
                        nc.tensor.matmul(
                            v_ps[:, o0:o0 + 256],
                            lhsT=A2_sb[:, kp, :, :],
                            rhs=xt[:, 2 * kp:2 * kp + 2, o0:o0 + 256],
                            start=(kp == 0), stop=(kp == 2),
                            perf_mode=DR)
                if n % 2 == 0:
                    nc.scalar.copy(v_sb[0:32, :], v_ps)
                else:
                    nc.vector.tensor_scalar_mul(v_sb[0:32, :], v_ps, 1.0)

            def emit_d3(n, j):
                """delta chunks 3j..3j+2 of tile n -> int8 (+ store)."""
                v_sb = v_sbs[n]
                yt = yts[n]
                for c in range(3 * j, 3 * j + 3):
                    d_ps = dp.tile([128, TTK], f32, tag="d", name="d_ps")
                    nc.tensor.matmul(d_ps, lhsT=Bc_sb[:, c, :],
                                     rhs=v_sb[:, :], start=True, stop=True)
                    dst = yt[:, c, :]
                    if (c + n) % 2 == 0:
                        nc.vector.tensor_scalar_mul(dst, d_ps, 1.0)
                    else:
                        nc.scalar.copy(dst, d_ps)
                if n == NT - 1:
                    # final tile: store each chunk triplet as soon as its
                    # conversions land, on parallel rings
                    ring = nc.gpsimd if j == 0 else nc.sync
                    w0 = 3 * j * TTK
                    ring.dma_start(
                        out=yP_r[:, n, w0:w0 + 3 * TTK],
                        in_=yt[:, 3 * j:3 * j + 3, :].rearrange(
                            "p c t -> p (c t)"))
                elif j == 1:
                    nc.gpsimd.dma_start(
                        out=yP_r[:, n, :],
                        in_=yt.rearrange("p c t -> p (c t)"))

            # software pipeline: v of tile n+2 interleaves with the delta
            # triplets of tile n
            emit_vq(0)
            emit_vq(1)
            for n in range(NT):
                emit_d3(n, 0)
                if n + 2 < NT:
                    emit_vq(n + 2)
                emit_d3(n, 1)

    nc.compile()
    return nc


def _get_program():
    global _PROGRAM
    if _PROGRAM is None:
        _PROGRAM = _build_program()
    return _PROGRAM


def _routing(inputs):
    """Host-side routing: returns Acomb [16,H] f32 (scaled), Bcomb [H,16]."""
    cur = np.asarray(inputs["task_embedding"], np.float32)
    la = np.asarray(inputs["loras_a"], np.float32)
    lb = np.asarray(inputs["loras_b"], np.float32)
    te = np.asarray(inputs["task_embeds"], np.float32)
    W1 = np.asarray(inputs["W1"], np.float32)
    W2 = np.asarray(inputs["W2"], np.float32)
    W3 = np.asarray(inputs["W3"], np.float32)
    W4 = np.asarray(inputs["W4"], np.float32)
    b1 = np.asarray(inputs["b1"], np.float32)
    b2 = np.asarray(inputs["b2"], np.float32)
    b3 = np.asarray(inputs["b3"], np.float32)
    b4 = np.asarray(inputs["b4"], np.float32)
    tid = int(np.asarray(inputs["current_task_id"]))

    cur_norm = np.linalg.norm(cur)
    emb_norms = np.linalg.norm(te, axis=-1)
    cos_sim = (te @ cur) / np.maximum(emb_norms * cur_norm, 1e-8)
    euclid = np.linalg.norm(te - cur[None, :], axis=-1)
    euclid_sim = 1.0 / (1.0 + euclid)
    comb = np.concatenate([np.broadcast_to(cur, te.shape), te], axis=-1)
    h = np.maximum(comb @ W1.T + b1, 0.0)
    h = np.maximum(h @ W2.T + b2, 0.0)
    h = np.maximum(h @ W3.T + b3, 0.0)
    nn_sim = 1.0 / (1.0 + np.exp(-(h @ W4.T + b4)))[..., 0]
    sims = 0.4 * cos_sim + 0.3 * euclid_sim + 0.3 * nn_sim

    top_idx = np.argpartition(-sims, TOP_K)[:TOP_K]
    top_vals = sims[top_idx]
    w = np.where(top_vals > 0.0, top_vals, 0.0)
    tw = float(w.sum())
    sw = tw if tw > 0 else 1.0
    pa = np.einsum('k,krh->rh', w, la[top_idx]) / sw
    pb = np.einsum('k,khr->hr', w, lb[top_idx]) / sw
    fw = min(cur_norm * 0.1, 0.5)
    c_cur = (1.0 - fw) * SCALING if tw > 0 else SCALING
    c_pool = fw * SCALING if tw > 0 else 0.0
    Acomb = np.concatenate([la[tid] * c_cur, pa * c_pool], axis=0)  # [16, H]
    Bcomb = np.concatenate([lb[tid], pb], axis=1)                   # [H, 16]
    return Acomb, Bcomb


def _make_in_maps(inputs):
    import ml_dtypes
    bf16 = ml_dtypes.bfloat16
    fp8 = ml_dtypes.float8_e4m3

    hs = np.asarray(inputs["hidden_states"], np.float32)
    Acomb, Bcomb = _routing(inputs)

    # A2[p, kp, i, m] = (KA*Acomb)[m, (2kp+i)*128 + p] for m<16, 0 pad to 32
    As = np.zeros((32, 768), np.float32)
    As[0:16] = Acomb * KA
    A2 = np.ascontiguousarray(
        As.astype(fp8).reshape(32, 3, 2, 128).transpose(3, 1, 2, 0))
    # Bpack[k, c, m] = Bdev[c*128+m, k], bf16, rows k>=16 zero
    Bdev = (Bcomb / (KA * S_D)).astype(bf16)          # [768, 16]
    Bpack = np.ascontiguousarray(
        Bdev.reshape(6, 128, 16).transpose(2, 0, 1))  # [16, 6, 128]

    wblob = np.zeros((128, _NBLOB), np.float32)
    wblob[:, 0:48] = A2.reshape(128, 192).view(np.float32)
    wblob[0:16, 48:48 + 384] = Bpack.reshape(16, 768).view(np.float32)

    x2 = hs.reshape(B * S, H)
    in_maps = []
    for i in range(NCORES):
        shard = np.ascontiguousarray(
            x2[i * TPC:(i + 1) * TPC].T).astype(fp8)   # [H=c*128+p, tok]
        # tile-major pack: [c,p,n,t] -> [n,p,c,t] -> [NT*128, 3072B]
        xpk = np.ascontiguousarray(
            shard.reshape(NCH, 128, NT, TTK).transpose(2, 1, 0, 3)
        ).reshape(NT * 128, NCH * TTK)
        in_maps.append({"xP": xpk.view(np.float32), "wblob": wblob})
    return in_maps


def kernel(**inputs):
    from concourse.bass_utils import run_bass_kernel_spmd

    nc = _get_program()
    in_maps = _make_in_maps(inputs)
    res = run_bass_kernel_spmd(nc, in_maps, core_ids=list(range(NCORES)))
    hs = np.asarray(inputs["hidden_states"], np.float32)
    out = np.empty((B * S, H), np.float32)
    x2 = hs.reshape(B * S, H)
    for i, r in enumerate(res.results):
        dpk = r["yP"].reshape(NT, 128, NCH, TTK).transpose(2, 1, 0, 3)
        dT = dpk.reshape(H, TPC)                       # [c*128+p, tok]
        out[i * TPC:(i + 1) * TPC] = (
            x2[i * TPC:(i + 1) * TPC] + dT.T.astype(np.float32) * S_D)
    return out.reshape(B, S, H)
